# revision 22
# baseline (speedup 1.0000x reference)
"""Trainium2 Bass kernel for windowed attention with dynamic position bias.

Shapes (hardcoded): qkv [3, 2, 65536, 192], H=W=256, window 8x32 (N=256),
6 heads, head_dim 32. 512 windows total, data-parallel over 8 cores
(64 windows each; each core owns a contiguous band of 64 H-rows of one batch).

Per-window device pipeline (v3):
  Host pre-packs Q^T/K^T per window in fp16 (no on-device transposes);
  q pre-scaled by a = SCALE*log2e*1024.
  Scores S^T[k,q] per (head, kk): K=32-contraction matmuls, 3-way row-packed
  via tile_position (base partitions 0/32/64), fp32 PSUM.
  heads 0-2 (buf A): bias pre-accumulated into PSUM by identity matmuls
    (start=True) streaming biasT = 1024*log2e*bias; scores accumulate on
    top (start=False); ACT exp (scale=ln2/1024) -> fp16 P, split into two
    region ops ([0:512], [512:1536]) so next window's identity rewrite of
    each region only waits on that region's exp (subtile deps).
  heads 3-5 (buf B): DVE Schraudolph exp2 bit-trick in one op:
    int16 = s' + B'[k,q] (B' folds bias + fp16 exponent offset + phase c),
    reinterpreted as fp16. The constant 2^-c factor cancels in softmax.
  AV: stationary P^T chunk [128,128] fp16, moving vext [128,34] ([V | 1]),
  out [q, (h, d+denom)] in fp32 PSUM; extraction copies (ACT + DVE) to fp16.
  Normalization (divide by denom) on host.
"""
import sys
import numpy as np

sys.path.insert(0, "/opt/trn_rl_repo")

H_SP, W_SP = 8, 32
NUM_HEADS = 6
DIM = 192
HEAD_DIM = 32
N = H_SP * W_SP          # 256 tokens per window
LN_EPS = 1e-5
SCALE = HEAD_DIM ** -0.5
B, H, W = 2, 256, 256
L = H * W
N_CORES = 8
WINDOWS_PER_CORE = 64    # 8 hb bands x 8 wi
L_PER_CORE = L // 4      # 16384 tokens (64 H-rows)

LOG2E = 1.4426950408889634
A_PRE = SCALE * LOG2E * 1024.0            # folded into qT on host
EXP_SCALE = float(np.log(2.0) / 1024.0)   # ACT path: exp(EXP_SCALE * s')
C_SHIFT = 0.04344                          # Schraudolph phase constant

_BUILT = None


def _np_layer_norm(x, g, b):
    m = x.mean(axis=-1, keepdims=True)
    v = ((x - m) ** 2).mean(axis=-1, keepdims=True)
    return (x - m) / np.sqrt(v + LN_EPS) * g + b


def _host_bias(rpi, rpe_biases, p):
    """DynamicPosBias MLP + gather -> rel [q, k, h] float32."""
    x = rpe_biases.astype(np.float32)
    pos = x @ p["pos_proj_w"].T + p["pos_proj_b"]
    pos = np.maximum(_np_layer_norm(pos, p["ln1_g"], p["ln1_b"]), 0.0) @ p["fc1_w"].T + p["fc1_b"]
    pos = np.maximum(_np_layer_norm(pos, p["ln2_g"], p["ln2_b"]), 0.0) @ p["fc2_w"].T + p["fc2_b"]
    pos = np.maximum(_np_layer_norm(pos, p["ln3_g"], p["ln3_b"]), 0.0) @ p["fc3_w"].T + p["fc3_b"]
    rel = pos[np.asarray(rpi).reshape(-1)].reshape(N, N, NUM_HEADS)  # [q, k, h]
    return rel


def _build():
    import concourse.bass as bass
    import concourse.mybir as mybir
    import concourse.tile as tile
    from concourse import bacc

    dt = mybir.dt
    fn = mybir.ActivationFunctionType
    alu = mybir.AluOpType
    nc = bacc.Bacc("TRN2", target_bir_lowering=False, debug=False)

    kT_c = nc.declare_dram_parameter("kT_c", [WINDOWS_PER_CORE, 96, 512], dt.float16, isOutput=False)
    qT_c = nc.declare_dram_parameter("qT_c", [WINDOWS_PER_CORE, 96, 512], dt.float16, isOutput=False)
    vext_c = nc.declare_dram_parameter("vext_c", [WINDOWS_PER_CORE, 128, 408], dt.float16, isOutput=False)
    biasT_c = nc.declare_dram_parameter("biasT_c", [128, 1024], dt.float16, isOutput=False)
    expb0_c = nc.declare_dram_parameter("expb0_c", [128, 512], dt.float16, isOutput=False)
    bprime_c = nc.declare_dram_parameter("bprime_c", [128, 1536], dt.float32, isOutput=False)
    ident_c = nc.declare_dram_parameter("ident_c", [128, 128], dt.float16, isOutput=False)
    out_c = nc.declare_dram_parameter("out_c", [WINDOWS_PER_CORE, 128, 408], dt.float16, isOutput=True)

    with tile.TileContext(nc) as tc:
        with (
            tc.tile_pool(name="const", bufs=1) as cp,
            tc.tile_pool(name="io", bufs=3) as io,
            tc.tile_pool(name="vp", bufs=3) as vp,
            tc.tile_pool(name="pt", bufs=2) as ptp,
            tc.tile_pool(name="op", bufs=3) as op,
            tc.tile_pool(name="ps_s", bufs=2, space="PSUM") as ps_s,
            tc.tile_pool(name="ps_av", bufs=2, space="PSUM") as ps_av,
        ):
            biasT = cp.tile([128, 1024], dt.float16, tag="biasT")
            nc.sync.dma_start(out=biasT[:], in_=biasT_c[:])
            bprime = cp.tile([128, 1536], dt.float32, tag="bprime")
            nc.scalar.dma_start(out=bprime[:], in_=bprime_c[:])
            ident = cp.tile([128, 128], dt.float16, tag="ident")
            nc.scalar.dma_start(out=ident[:], in_=ident_c[:])
            expb0 = cp.tile([128, 512], dt.float16, tag="expb0")
            nc.sync.dma_start(out=expb0[:], in_=expb0_c[:])

            state = {}
            avstate = {}

            def emit_front(w):
                qT = io.tile([96, 512], dt.float16, tag="qT")
                kT = io.tile([96, 512], dt.float16, tag="kT")
                vext = vp.tile([128, 408], dt.float16, tag="vext")
                nc.sync.dma_start(out=qT[:], in_=qT_c[w])
                nc.sync.dma_start(out=kT[:], in_=kT_c[w])
                nc.gpsimd.dma_start(out=vext[:], in_=vext_c[w])

                ptb = ptp.tile([128, 3072], dt.float16, tag="ptb")
                # Double-buffered [128,1536] tiles; ACT half alternates
                # buffers by parity. exp is split into two region ops so each
                # region's identity rewrite next window only waits on that
                # region's exp (subtile dependency tracking).
                t1 = ps_s.tile([128, 1536], dt.float32, tag="scores")
                t2 = ps_s.tile([128, 1536], dt.float32, tag="scores")
                ps_act, ps_schr = (t1, t2) if w % 2 == 0 else (t2, t1)

                # bias for heads 1-2 via identity matmul; head 0 gets
                # bias as a DVE exp(bias) multiply after exp_A instead,
                # removing one identity MM from the critical chain.
                for c0 in (0, 512):
                    nc.tensor.matmul(
                        ps_act[:, 512 + c0: 1024 + c0], ident[:],
                        biasT[:, c0:c0 + 512],
                        start=True, stop=False, skip_group_check=True)
                for kk in (0, 1):
                    for s in (0, 1, 2):
                        nc.tensor.matmul(
                            ps_act[:, s * 512 + kk * 256: s * 512 + kk * 256 + 256],
                            kT[32 * s:32 * s + 32, kk * 128: kk * 128 + 128],
                            qT[32 * s:32 * s + 32, 0:256],
                            start=(s == 0), stop=True, skip_group_check=True)
                pt0 = ptp.tile([128, 512], dt.float16, tag="pt0")
                nc.scalar.activation(pt0[:], ps_act[:, 0:512],
                                     fn.Exp, scale=EXP_SCALE)
                nc.scalar.activation(ptb[:, 512:1536], ps_act[:, 512:1536],
                                     fn.Exp, scale=EXP_SCALE)
                # Schraudolph half (heads 3-5): 2 waves of 3 row-packed MMs
                for kk in (0, 1):
                    for s in (0, 1, 2):
                        nc.tensor.matmul(
                            ps_schr[:, s * 512 + kk * 256: s * 512 + kk * 256 + 256],
                            kT[32 * s:32 * s + 32,
                               256 + kk * 128: 256 + kk * 128 + 128],
                            qT[32 * s:32 * s + 32, 256:512],
                            start=True, stop=True, skip_group_check=True)
                nc.vector.tensor_tensor(
                    out=ptb[:, 1536:3072].bitcast(dt.int16),
                    in0=ps_schr[:], in1=bprime[:], op=alu.add)
                nc.vector.tensor_tensor(out=ptb[:, 0:512], in0=pt0[:],
                                        in1=expb0[:], op=alu.mult)
                state[w] = (ptb, vext)

            def emit_av(w):
                ptb, vext = state.pop(w)
                psav = ps_av.tile([128, 408], dt.float32, tag="av")
                for qc in (0, 1):
                    for h in range(6):
                        for kk in (0, 1):
                            nc.tensor.matmul(
                                psav[:, qc * 204 + h * 34: qc * 204 + h * 34 + 34],
                                ptb[:, h * 512 + kk * 256 + qc * 128:
                                    h * 512 + kk * 256 + qc * 128 + 128],
                                vext[:, kk * 204 + h * 34: kk * 204 + h * 34 + 34],
                                start=(kk == 0), stop=(kk == 1),
                                skip_group_check=True)
                avstate[w] = psav

            def emit_extract(w):
                psav = avstate.pop(w)
                osb = op.tile([128, 408], dt.float16, tag="osb")
                nc.scalar.copy(osb[:, 0:204], psav[:, 0:204])
                nc.vector.tensor_copy(osb[:, 204:408], psav[:, 204:408])
                nc.gpsimd.dma_start(out=out_c[w], in_=osb[:])

            for w in range(WINDOWS_PER_CORE + 2):
                if w >= 2:
                    emit_extract(w - 2)
                if w < WINDOWS_PER_CORE:
                    emit_front(w)
                if 1 <= w <= WINDOWS_PER_CORE:
                    emit_av(w - 1)
    nc.compile()
    return nc


def _get_nc():
    global _BUILT
    if _BUILT is None:
        _BUILT = _build()
    return _BUILT


def _pack_qkT(x, scale):
    """x: [L_PER_CORE, 192] fp32 -> [64, 96, 512] fp16 window-packed c-major.

    Window token t = (h_row, j); c = half*96 + (32*s + d).
    Output[win, 32s+d, half*256 + t].
    """
    v = x.reshape(8, 8, 8, 32, DIM)                # [hb, h, wi, j, c]
    v = v.transpose(0, 2, 4, 1, 3)                 # [hb, wi, c, h, j]
    v = v.reshape(64, DIM, 256)                    # [win, c, t]
    v = v.reshape(64, 2, 96, 256).transpose(0, 2, 1, 3).reshape(64, 96, 512)
    if scale != 1.0:
        v = v * np.float32(scale)
    return np.ascontiguousarray(v.astype(np.float16))


def kernel(qkv, H, W, rpi, rpe_biases, pos_proj_w, pos_proj_b, ln1_g, ln1_b,
           fc1_w, fc1_b, ln2_g, ln2_b, fc2_w, fc2_b, ln3_g, ln3_b,
           fc3_w, fc3_b, _trace=False):
    from concourse.bass_utils import run_bass_kernel_spmd

    qkv = np.asarray(qkv, dtype=np.float32)
    params = dict(pos_proj_w=pos_proj_w, pos_proj_b=pos_proj_b, ln1_g=ln1_g,
                  ln1_b=ln1_b, fc1_w=fc1_w, fc1_b=fc1_b, ln2_g=ln2_g,
                  ln2_b=ln2_b, fc2_w=fc2_w, fc2_b=fc2_b, ln3_g=ln3_g,
                  ln3_b=ln3_b, fc3_w=fc3_w, fc3_b=fc3_b)
    params = {k: np.asarray(v, dtype=np.float32) for k, v in params.items()}
    rel = _host_bias(rpi, rpe_biases, params)      # [q, k, h] fp32

    # biasT (fp16) for ACT heads 1-2: 1024*log2e*bias
    biasT = np.empty((128, 1024), dtype=np.float16)
    for i, h in enumerate((1, 2)):
        for kk in range(2):
            blk = 1024.0 * LOG2E * rel[:, kk * 128:(kk + 1) * 128, h].T
            biasT[:, i * 512 + kk * 256: i * 512 + kk * 256 + 256] = blk
    # head 0: multiplicative exp(bias), applied post-exp on DVE
    expb0 = np.empty((128, 512), dtype=np.float16)
    for kk in range(2):
        expb0[:, kk * 256:(kk + 1) * 256] = np.exp(
            rel[:, kk * 128:(kk + 1) * 128, 0].T)
    # B' fp32 for Schraudolph heads 3-5: 1024*(log2e*bias + 15 - c)
    bprime = np.empty((128, 1536), dtype=np.float32)
    for i, h in enumerate(range(3, 6)):
        for kk in range(2):
            blk = 1024.0 * (LOG2E * rel[:, kk * 128:(kk + 1) * 128, h].T
                            + 15.0 - C_SHIFT)
            bprime[:, i * 512 + kk * 256: i * 512 + kk * 256 + 256] = blk
    ident = np.eye(128, dtype=np.float16)

    nc = _get_nc()
    in_maps = []
    for c in range(N_CORES):
        b = c // 4
        row0 = (c % 4) * L_PER_CORE
        qT = _pack_qkT(qkv[0, b, row0:row0 + L_PER_CORE], A_PRE)
        kT = _pack_qkT(qkv[1, b, row0:row0 + L_PER_CORE], 1.0)
        vc = qkv[2, b, row0:row0 + L_PER_CORE].reshape(8, 8, 8, 32, DIM)
        win = vc.transpose(0, 2, 1, 3, 4).reshape(64, 2, 128, 6, 32)
        tmp = np.zeros((64, 2, 128, 6, 34), dtype=np.float32)
        tmp[..., :32] = win
        tmp[..., 32] = 1.0
        vext = np.ascontiguousarray(
            tmp.transpose(0, 2, 1, 3, 4).reshape(64, 128, 408).astype(np.float16))
        in_maps.append({
            "qT_c": qT,
            "kT_c": kT,
            "vext_c": vext,
            "biasT_c": biasT,
            "expb0_c": expb0,
            "bprime_c": bprime,
            "ident_c": ident,
        })
    res = run_bass_kernel_spmd(nc, in_maps, list(range(N_CORES)), trace=_trace)

    out = np.empty((B, H, W, DIM), dtype=np.float32)
    for c in range(N_CORES):
        b = c // 4
        h0 = (c % 4) * 64
        raw = res.results[c]["out_c"].astype(np.float32)  # [64, 128, 408]
        raw = raw.reshape(64, 128, 2, 6, 34)              # [win, p, qc, h, d]
        num = raw[..., :32]
        den = raw[..., 32]
        o = num / den[..., None]                          # [win, p, qc, h, 32]
        o = o.transpose(0, 2, 1, 3, 4).reshape(64, 256, DIM)  # [win, t, c]
        o = o.reshape(8, 8, 8, 32, DIM).transpose(0, 2, 1, 3, 4)
        out[b, h0:h0 + 64] = o.reshape(64, W, DIM)
    if _trace:
        return out, res
    return out


# revision 24
# speedup vs baseline: 1.2005x; 1.2005x over previous
"""Trainium2 Bass kernel for windowed attention with dynamic position bias.

Shapes (hardcoded): qkv [3, 2, 65536, 192], H=W=256, window 8x32 (N=256),
6 heads, head_dim 32. 512 windows total, data-parallel over 8 cores
(64 windows each; each core owns a contiguous band of 64 H-rows of one batch).

Per-window device pipeline (v3):
  Host pre-packs Q^T/K^T per window in fp16 (no on-device transposes);
  q pre-scaled by a = SCALE*log2e*1024.
  Scores S^T[k,q] per (head, kk): K=32-contraction matmuls, 3-way row-packed
  via tile_position (base partitions 0/32/64), fp32 PSUM.
  heads 0-2 (buf A): bias pre-accumulated into PSUM by identity matmuls
    (start=True) streaming biasT = 1024*log2e*bias; scores accumulate on
    top (start=False); ACT exp (scale=ln2/1024) -> fp16 P, split into two
    region ops ([0:512], [512:1536]) so next window's identity rewrite of
    each region only waits on that region's exp (subtile deps).
  heads 3-5 (buf B): DVE Schraudolph exp2 bit-trick in one op:
    int16 = s' + B'[k,q] (B' folds bias + fp16 exponent offset + phase c),
    reinterpreted as fp16. The constant 2^-c factor cancels in softmax.
  AV: stationary P^T chunk [128,128] fp16, moving vext [128,34] ([V | 1]),
  out [q, (h, d+denom)] in fp32 PSUM; extraction copies (ACT + DVE) to fp16.
  Normalization (divide by denom) on host.
"""
import sys
import numpy as np

sys.path.insert(0, "/opt/trn_rl_repo")

H_SP, W_SP = 8, 32
NUM_HEADS = 6
DIM = 192
HEAD_DIM = 32
N = H_SP * W_SP          # 256 tokens per window
LN_EPS = 1e-5
SCALE = HEAD_DIM ** -0.5
B, H, W = 2, 256, 256
L = H * W
N_CORES = 8
WINDOWS_PER_CORE = 64    # 8 hb bands x 8 wi
L_PER_CORE = L // 4      # 16384 tokens (64 H-rows)

LOG2E = 1.4426950408889634
A_PRE = SCALE * LOG2E * 1024.0            # folded into qT on host
EXP_SCALE = float(np.log(2.0) / 1024.0)   # ACT path: exp(EXP_SCALE * s')
C_SHIFT = 0.04344                          # Schraudolph phase constant

_BUILT = None


def _np_layer_norm(x, g, b):
    m = x.mean(axis=-1, keepdims=True)
    v = ((x - m) ** 2).mean(axis=-1, keepdims=True)
    return (x - m) / np.sqrt(v + LN_EPS) * g + b


def _host_bias(rpi, rpe_biases, p):
    """DynamicPosBias MLP + gather -> rel [q, k, h] float32."""
    x = rpe_biases.astype(np.float32)
    pos = x @ p["pos_proj_w"].T + p["pos_proj_b"]
    pos = np.maximum(_np_layer_norm(pos, p["ln1_g"], p["ln1_b"]), 0.0) @ p["fc1_w"].T + p["fc1_b"]
    pos = np.maximum(_np_layer_norm(pos, p["ln2_g"], p["ln2_b"]), 0.0) @ p["fc2_w"].T + p["fc2_b"]
    pos = np.maximum(_np_layer_norm(pos, p["ln3_g"], p["ln3_b"]), 0.0) @ p["fc3_w"].T + p["fc3_b"]
    rel = pos[np.asarray(rpi).reshape(-1)].reshape(N, N, NUM_HEADS)  # [q, k, h]
    return rel


def _build():
    import concourse.bass as bass
    import concourse.mybir as mybir
    import concourse.tile as tile
    from concourse import bacc

    dt = mybir.dt
    fn = mybir.ActivationFunctionType
    alu = mybir.AluOpType
    nc = bacc.Bacc("TRN2", target_bir_lowering=False, debug=False)

    kT_c = nc.declare_dram_parameter("kT_c", [WINDOWS_PER_CORE, 96, 512], dt.float16, isOutput=False)
    qT_c = nc.declare_dram_parameter("qT_c", [WINDOWS_PER_CORE, 96, 512], dt.float16, isOutput=False)
    vext_c = nc.declare_dram_parameter("vext_c", [WINDOWS_PER_CORE, 128, 408], dt.float16, isOutput=False)
    biasT_c = nc.declare_dram_parameter("biasT_c", [128, 1536], dt.float16, isOutput=False)
    bprime_c = nc.declare_dram_parameter("bprime_c", [128, 1536], dt.float32, isOutput=False)
    ident_c = nc.declare_dram_parameter("ident_c", [128, 128], dt.float16, isOutput=False)
    out_c = nc.declare_dram_parameter("out_c", [WINDOWS_PER_CORE, 128, 408], dt.float16, isOutput=True)

    with tile.TileContext(nc) as tc:
        with (
            tc.tile_pool(name="const", bufs=1) as cp,
            tc.tile_pool(name="io", bufs=3) as io,
            tc.tile_pool(name="vp", bufs=3) as vp,
            tc.tile_pool(name="pt", bufs=2) as ptp,
            tc.tile_pool(name="op", bufs=3) as op,
            tc.tile_pool(name="ps_s", bufs=2, space="PSUM") as ps_s,
            tc.tile_pool(name="ps_av", bufs=2, space="PSUM") as ps_av,
        ):
            biasT = cp.tile([128, 1536], dt.float16, tag="biasT")
            nc.sync.dma_start(out=biasT[:], in_=biasT_c[:])
            bprime = cp.tile([128, 1536], dt.float32, tag="bprime")
            nc.sync.dma_start(out=bprime[:], in_=bprime_c[:])
            ident = cp.tile([128, 128], dt.float16, tag="ident")
            nc.sync.dma_start(out=ident[:], in_=ident_c[:])

            state = {}
            avstate = {}

            def emit_front(w):
                qT = io.tile([96, 512], dt.float16, tag="qT")
                kT = io.tile([96, 512], dt.float16, tag="kT")
                vext = vp.tile([128, 408], dt.float16, tag="vext")
                nc.sync.dma_start(out=qT[:], in_=qT_c[w])
                nc.sync.dma_start(out=kT[:], in_=kT_c[w])
                nc.gpsimd.dma_start(out=vext[:], in_=vext_c[w])

                ptb = ptp.tile([128, 3072], dt.float16, tag="ptb")
                # Double-buffered [128,1536] tiles; ACT half alternates
                # buffers by parity. exp is split into two region ops so each
                # region's identity rewrite next window only waits on that
                # region's exp (subtile dependency tracking).
                t1 = ps_s.tile([128, 1536], dt.float32, tag="scores")
                t2 = ps_s.tile([128, 1536], dt.float32, tag="scores")
                ps_act, ps_schr = (t1, t2) if w % 2 == 0 else (t2, t1)

                # bias preloaded for ACT heads via identity matmul
                for c0 in (512, 1024, 0):
                    nc.tensor.matmul(
                        ps_act[:, c0:c0 + 512], ident[:], biasT[:, c0:c0 + 512],
                        start=True, stop=False, skip_group_check=True)
                for kk in (0, 1):
                    for s in (0, 1, 2):
                        nc.tensor.matmul(
                            ps_act[:, s * 512 + kk * 256: s * 512 + kk * 256 + 256],
                            kT[32 * s:32 * s + 32, kk * 128: kk * 128 + 128],
                            qT[32 * s:32 * s + 32, 0:256],
                            start=False, stop=True, skip_group_check=True)
                nc.scalar.activation(ptb[:, 0:1536], ps_act[:],
                                     fn.Exp, scale=EXP_SCALE)
                # Schraudolph half (heads 3-5): 2 waves of 3 row-packed MMs
                for kk in (0, 1):
                    for s in (0, 1, 2):
                        nc.tensor.matmul(
                            ps_schr[:, s * 512 + kk * 256: s * 512 + kk * 256 + 256],
                            kT[32 * s:32 * s + 32,
                               256 + kk * 128: 256 + kk * 128 + 128],
                            qT[32 * s:32 * s + 32, 256:512],
                            start=True, stop=True, skip_group_check=True)
                nc.vector.tensor_tensor(
                    out=ptb[:, 1536:3072].bitcast(dt.int16),
                    in0=ps_schr[:], in1=bprime[:], op=alu.add)
                state[w] = (ptb, vext)

            def emit_av(w):
                ptb, vext = state.pop(w)
                psav = ps_av.tile([128, 408], dt.float32, tag="av")
                for qc in (0, 1):
                    for h in range(6):
                        for kk in (0, 1):
                            nc.tensor.matmul(
                                psav[:, qc * 204 + h * 34: qc * 204 + h * 34 + 34],
                                ptb[:, h * 512 + kk * 256 + qc * 128:
                                    h * 512 + kk * 256 + qc * 128 + 128],
                                vext[:, kk * 204 + h * 34: kk * 204 + h * 34 + 34],
                                start=(kk == 0), stop=(kk == 1),
                                skip_group_check=True)
                avstate[w] = psav

            def emit_extract(w):
                psav = avstate.pop(w)
                osb = op.tile([128, 408], dt.float16, tag="osb")
                nc.scalar.copy(osb[:, 0:204], psav[:, 0:204])
                nc.vector.tensor_copy(osb[:, 204:408], psav[:, 204:408])
                nc.gpsimd.dma_start(out=out_c[w], in_=osb[:])

            for w in range(WINDOWS_PER_CORE + 2):
                if w >= 2:
                    emit_extract(w - 2)
                if w < WINDOWS_PER_CORE:
                    emit_front(w)
                if 1 <= w <= WINDOWS_PER_CORE:
                    emit_av(w - 1)
    nc.compile()
    return nc


def _get_nc():
    global _BUILT
    if _BUILT is None:
        _BUILT = _build()
    return _BUILT


def _pack_qkT(x, scale):
    """x: [L_PER_CORE, 192] fp32 -> [64, 96, 512] fp16 window-packed c-major.

    Window token t = (h_row, j); c = half*96 + (32*s + d).
    Output[win, 32s+d, half*256 + t].
    """
    v = x.reshape(8, 8, 8, 32, DIM)                # [hb, h, wi, j, c]
    v = v.transpose(0, 2, 4, 1, 3)                 # [hb, wi, c, h, j]
    v = v.reshape(64, DIM, 256)                    # [win, c, t]
    v = v.reshape(64, 2, 96, 256).transpose(0, 2, 1, 3).reshape(64, 96, 512)
    if scale != 1.0:
        v = v * np.float32(scale)
    return np.ascontiguousarray(v.astype(np.float16))


def kernel(qkv, H, W, rpi, rpe_biases, pos_proj_w, pos_proj_b, ln1_g, ln1_b,
           fc1_w, fc1_b, ln2_g, ln2_b, fc2_w, fc2_b, ln3_g, ln3_b,
           fc3_w, fc3_b, _trace=False):
    from concourse.bass_utils import run_bass_kernel_spmd

    qkv = np.asarray(qkv, dtype=np.float32)
    params = dict(pos_proj_w=pos_proj_w, pos_proj_b=pos_proj_b, ln1_g=ln1_g,
                  ln1_b=ln1_b, fc1_w=fc1_w, fc1_b=fc1_b, ln2_g=ln2_g,
                  ln2_b=ln2_b, fc2_w=fc2_w, fc2_b=fc2_b, ln3_g=ln3_g,
                  ln3_b=ln3_b, fc3_w=fc3_w, fc3_b=fc3_b)
    params = {k: np.asarray(v, dtype=np.float32) for k, v in params.items()}
    rel = _host_bias(rpi, rpe_biases, params)      # [q, k, h] fp32

    # biasT (fp16) for ACT heads 0-2: 1024*log2e*bias, layout [k, h*512+kk*256+q]
    biasT = np.empty((128, 1536), dtype=np.float16)
    for h in range(3):
        for kk in range(2):
            blk = 1024.0 * LOG2E * rel[:, kk * 128:(kk + 1) * 128, h].T
            biasT[:, h * 512 + kk * 256: h * 512 + kk * 256 + 256] = blk
    # B' fp32 for Schraudolph heads 3-5: 1024*(log2e*bias + 15 - c)
    bprime = np.empty((128, 1536), dtype=np.float32)
    for i, h in enumerate(range(3, 6)):
        for kk in range(2):
            blk = 1024.0 * (LOG2E * rel[:, kk * 128:(kk + 1) * 128, h].T
                            + 15.0 - C_SHIFT)
            bprime[:, i * 512 + kk * 256: i * 512 + kk * 256 + 256] = blk
    ident = np.eye(128, dtype=np.float16)

    nc = _get_nc()
    in_maps = []
    for c in range(N_CORES):
        b = c // 4
        row0 = (c % 4) * L_PER_CORE
        qT = _pack_qkT(qkv[0, b, row0:row0 + L_PER_CORE], A_PRE)
        kT = _pack_qkT(qkv[1, b, row0:row0 + L_PER_CORE], 1.0)
        vc = qkv[2, b, row0:row0 + L_PER_CORE].reshape(8, 8, 8, 32, DIM)
        win = vc.transpose(0, 2, 1, 3, 4).reshape(64, 2, 128, 6, 32)
        tmp = np.zeros((64, 2, 128, 6, 34), dtype=np.float32)
        tmp[..., :32] = win
        tmp[..., 32] = 1.0
        vext = np.ascontiguousarray(
            tmp.transpose(0, 2, 1, 3, 4).reshape(64, 128, 408).astype(np.float16))
        in_maps.append({
            "qT_c": qT,
            "kT_c": kT,
            "vext_c": vext,
            "biasT_c": biasT,
            "bprime_c": bprime,
            "ident_c": ident,
        })
    res = run_bass_kernel_spmd(nc, in_maps, list(range(N_CORES)), trace=_trace)

    out = np.empty((B, H, W, DIM), dtype=np.float32)
    for c in range(N_CORES):
        b = c // 4
        h0 = (c % 4) * 64
        raw = res.results[c]["out_c"].astype(np.float32)  # [64, 128, 408]
        raw = raw.reshape(64, 128, 2, 6, 34)              # [win, p, qc, h, d]
        num = raw[..., :32]
        den = raw[..., 32]
        o = num / den[..., None]                          # [win, p, qc, h, 32]
        o = o.transpose(0, 2, 1, 3, 4).reshape(64, 256, DIM)  # [win, t, c]
        o = o.reshape(8, 8, 8, 32, DIM).transpose(0, 2, 1, 3, 4)
        out[b, h0:h0 + 64] = o.reshape(64, W, DIM)
    if _trace:
        return out, res
    return out


# revision 26
# speedup vs baseline: 1.2108x; 1.0085x over previous
"""Trainium2 Bass kernel for windowed attention with dynamic position bias.

Shapes (hardcoded): qkv [3, 2, 65536, 192], H=W=256, window 8x32 (N=256),
6 heads, head_dim 32. 512 windows total, data-parallel over 8 cores
(64 windows each; each core owns a contiguous band of 64 H-rows of one batch).

Per-window device pipeline (v3):
  Host pre-packs Q^T/K^T per window in fp16 (no on-device transposes);
  q pre-scaled by a = SCALE*log2e*1024.
  Scores S^T[k,q] per (head, kk): K=32-contraction matmuls, 3-way row-packed
  via tile_position (base partitions 0/32/64), fp32 PSUM.
  heads 0-2 (buf A): bias pre-accumulated into PSUM by identity matmuls
    (start=True) streaming biasT = 1024*log2e*bias; scores accumulate on
    top (start=False); one ACT exp (scale=ln2/1024) -> fp16 P.
  heads 3-5 (buf B): DVE Schraudolph exp2 bit-trick in one op:
    int16 = s' + B'[k,q] (B' folds bias + fp16 exponent offset + phase c),
    reinterpreted as fp16. The constant 2^-c factor cancels in softmax.
  AV: stationary P^T chunk [128,128] fp16, moving vext [128,34] ([V | 1]),
  out [q, (h, d+denom)] in fp32 PSUM; extraction copies (ACT + DVE) to fp16.
  Normalization (divide by denom) on host.
"""
import sys
import numpy as np

sys.path.insert(0, "/opt/trn_rl_repo")

H_SP, W_SP = 8, 32
NUM_HEADS = 6
DIM = 192
HEAD_DIM = 32
N = H_SP * W_SP          # 256 tokens per window
LN_EPS = 1e-5
SCALE = HEAD_DIM ** -0.5
B, H, W = 2, 256, 256
L = H * W
N_CORES = 8
WINDOWS_PER_CORE = 64    # 8 hb bands x 8 wi
L_PER_CORE = L // 4      # 16384 tokens (64 H-rows)

LOG2E = 1.4426950408889634
A_PRE = SCALE * LOG2E * 1024.0            # folded into qT on host
EXP_SCALE = float(np.log(2.0) / 1024.0)   # ACT path: exp(EXP_SCALE * s')
C_SHIFT = 0.04344                          # Schraudolph phase constant

_BUILT = None


def _np_layer_norm(x, g, b):
    m = x.mean(axis=-1, keepdims=True)
    v = ((x - m) ** 2).mean(axis=-1, keepdims=True)
    return (x - m) / np.sqrt(v + LN_EPS) * g + b


def _host_bias(rpi, rpe_biases, p):
    """DynamicPosBias MLP + gather -> rel [q, k, h] float32."""
    x = rpe_biases.astype(np.float32)
    pos = x @ p["pos_proj_w"].T + p["pos_proj_b"]
    pos = np.maximum(_np_layer_norm(pos, p["ln1_g"], p["ln1_b"]), 0.0) @ p["fc1_w"].T + p["fc1_b"]
    pos = np.maximum(_np_layer_norm(pos, p["ln2_g"], p["ln2_b"]), 0.0) @ p["fc2_w"].T + p["fc2_b"]
    pos = np.maximum(_np_layer_norm(pos, p["ln3_g"], p["ln3_b"]), 0.0) @ p["fc3_w"].T + p["fc3_b"]
    rel = pos[np.asarray(rpi).reshape(-1)].reshape(N, N, NUM_HEADS)  # [q, k, h]
    return rel


def _build():
    import concourse.bass as bass
    import concourse.mybir as mybir
    import concourse.tile as tile
    from concourse import bacc

    dt = mybir.dt
    fn = mybir.ActivationFunctionType
    alu = mybir.AluOpType
    nc = bacc.Bacc("TRN2", target_bir_lowering=False, debug=False)

    kT_c = nc.declare_dram_parameter("kT_c", [WINDOWS_PER_CORE, 96, 512], dt.float16, isOutput=False)
    qT_c = nc.declare_dram_parameter("qT_c", [WINDOWS_PER_CORE, 96, 512], dt.float16, isOutput=False)
    vext_c = nc.declare_dram_parameter("vext_c", [WINDOWS_PER_CORE, 128, 408], dt.float16, isOutput=False)
    biasT_c = nc.declare_dram_parameter("biasT_c", [128, 1536], dt.float16, isOutput=False)
    bprime_c = nc.declare_dram_parameter("bprime_c", [128, 1536], dt.float32, isOutput=False)
    ident_c = nc.declare_dram_parameter("ident_c", [128, 128], dt.float16, isOutput=False)
    out_c = nc.declare_dram_parameter("out_c", [WINDOWS_PER_CORE, 128, 408], dt.float16, isOutput=True)

    with tile.TileContext(nc) as tc:
        with (
            tc.tile_pool(name="const", bufs=1) as cp,
            tc.tile_pool(name="io", bufs=3) as io,
            tc.tile_pool(name="vp", bufs=3) as vp,
            tc.tile_pool(name="pt", bufs=2) as ptp,
            tc.tile_pool(name="op", bufs=3) as op,
            tc.tile_pool(name="ps_s", bufs=2, space="PSUM") as ps_s,
            tc.tile_pool(name="ps_av", bufs=2, space="PSUM") as ps_av,
        ):
            biasT = cp.tile([128, 1536], dt.float16, tag="biasT")
            nc.sync.dma_start(out=biasT[:], in_=biasT_c[:])
            bprime = cp.tile([128, 1536], dt.float32, tag="bprime")
            nc.scalar.dma_start(out=bprime[:], in_=bprime_c[:])
            ident = cp.tile([128, 128], dt.float16, tag="ident")
            nc.scalar.dma_start(out=ident[:], in_=ident_c[:])

            state = {}
            avstate = {}

            def emit_front(w):
                qT = io.tile([96, 512], dt.float16, tag="qT")
                kT = io.tile([96, 512], dt.float16, tag="kT")
                vext = vp.tile([128, 408], dt.float16, tag="vext")
                nc.sync.dma_start(out=qT[:], in_=qT_c[w])
                nc.sync.dma_start(out=kT[:], in_=kT_c[w])
                nc.gpsimd.dma_start(out=vext[:], in_=vext_c[w])

                ptb = ptp.tile([128, 3072], dt.float16, tag="ptb")
                # Double-buffered [128,1536] tiles; ACT half alternates
                # buffers by parity. exp is split into two region ops so each
                # region's identity rewrite next window only waits on that
                # region's exp (subtile dependency tracking).
                t1 = ps_s.tile([128, 1536], dt.float32, tag="scores")
                t2 = ps_s.tile([128, 1536], dt.float32, tag="scores")
                ps_act, ps_schr = (t1, t2) if w % 2 == 0 else (t2, t1)

                # bias preloaded for ACT heads via identity matmul
                for c0 in (512, 1024, 0):
                    nc.tensor.matmul(
                        ps_act[:, c0:c0 + 512], ident[:], biasT[:, c0:c0 + 512],
                        start=True, stop=False, skip_group_check=True)
                for kk in (0, 1):
                    for s in (0, 1, 2):
                        nc.tensor.matmul(
                            ps_act[:, s * 512 + kk * 256: s * 512 + kk * 256 + 256],
                            kT[32 * s:32 * s + 32, kk * 128: kk * 128 + 128],
                            qT[32 * s:32 * s + 32, 0:256],
                            start=False, stop=True, skip_group_check=True)
                nc.scalar.activation(ptb[:, 0:1536], ps_act[:],
                                     fn.Exp, scale=EXP_SCALE)
                # Schraudolph half (heads 3-5): 2 waves of 3 row-packed MMs
                for kk in (0, 1):
                    for s in (0, 1, 2):
                        nc.tensor.matmul(
                            ps_schr[:, s * 512 + kk * 256: s * 512 + kk * 256 + 256],
                            kT[32 * s:32 * s + 32,
                               256 + kk * 128: 256 + kk * 128 + 128],
                            qT[32 * s:32 * s + 32, 256:512],
                            start=True, stop=True, skip_group_check=True)
                nc.vector.tensor_tensor(
                    out=ptb[:, 1536:3072].bitcast(dt.int16),
                    in0=ps_schr[:], in1=bprime[:], op=alu.add)
                state[w] = (ptb, vext)

            def emit_av(w):
                ptb, vext = state.pop(w)
                psav = ps_av.tile([128, 408], dt.float32, tag="av")
                for qc in (0, 1):
                    for h in range(6):
                        for kk in (0, 1):
                            nc.tensor.matmul(
                                psav[:, qc * 204 + h * 34: qc * 204 + h * 34 + 34],
                                ptb[:, h * 512 + kk * 256 + qc * 128:
                                    h * 512 + kk * 256 + qc * 128 + 128],
                                vext[:, kk * 204 + h * 34: kk * 204 + h * 34 + 34],
                                start=(kk == 0), stop=(kk == 1),
                                skip_group_check=True)
                avstate[w] = psav

            def emit_extract(w):
                psav = avstate.pop(w)
                osb = op.tile([128, 408], dt.float16, tag="osb")
                nc.scalar.copy(osb[:, 0:204], psav[:, 0:204])
                nc.vector.tensor_copy(osb[:, 204:408], psav[:, 204:408])
                nc.gpsimd.dma_start(out=out_c[w], in_=osb[:])

            for w in range(WINDOWS_PER_CORE + 2):
                if w >= 2:
                    emit_extract(w - 2)
                if w < WINDOWS_PER_CORE:
                    emit_front(w)
                if 1 <= w <= WINDOWS_PER_CORE:
                    emit_av(w - 1)
    nc.compile()
    return nc


def _get_nc():
    global _BUILT
    if _BUILT is None:
        _BUILT = _build()
    return _BUILT


def _pack_qkT(x, scale):
    """x: [L_PER_CORE, 192] fp32 -> [64, 96, 512] fp16 window-packed c-major.

    Window token t = (h_row, j); c = half*96 + (32*s + d).
    Output[win, 32s+d, half*256 + t].
    """
    v = x.reshape(8, 8, 8, 32, DIM)                # [hb, h, wi, j, c]
    v = v.transpose(0, 2, 4, 1, 3)                 # [hb, wi, c, h, j]
    v = v.reshape(64, DIM, 256)                    # [win, c, t]
    v = v.reshape(64, 2, 96, 256).transpose(0, 2, 1, 3).reshape(64, 96, 512)
    if scale != 1.0:
        v = v * np.float32(scale)
    return np.ascontiguousarray(v.astype(np.float16))


def kernel(qkv, H, W, rpi, rpe_biases, pos_proj_w, pos_proj_b, ln1_g, ln1_b,
           fc1_w, fc1_b, ln2_g, ln2_b, fc2_w, fc2_b, ln3_g, ln3_b,
           fc3_w, fc3_b, _trace=False):
    from concourse.bass_utils import run_bass_kernel_spmd

    qkv = np.asarray(qkv, dtype=np.float32)
    params = dict(pos_proj_w=pos_proj_w, pos_proj_b=pos_proj_b, ln1_g=ln1_g,
                  ln1_b=ln1_b, fc1_w=fc1_w, fc1_b=fc1_b, ln2_g=ln2_g,
                  ln2_b=ln2_b, fc2_w=fc2_w, fc2_b=fc2_b, ln3_g=ln3_g,
                  ln3_b=ln3_b, fc3_w=fc3_w, fc3_b=fc3_b)
    params = {k: np.asarray(v, dtype=np.float32) for k, v in params.items()}
    rel = _host_bias(rpi, rpe_biases, params)      # [q, k, h] fp32

    # biasT (fp16) for ACT heads 0-2: 1024*log2e*bias, layout [k, h*512+kk*256+q]
    biasT = np.empty((128, 1536), dtype=np.float16)
    for h in range(3):
        for kk in range(2):
            blk = 1024.0 * LOG2E * rel[:, kk * 128:(kk + 1) * 128, h].T
            biasT[:, h * 512 + kk * 256: h * 512 + kk * 256 + 256] = blk
    # B' fp32 for Schraudolph heads 3-5: 1024*(log2e*bias + 15 - c)
    bprime = np.empty((128, 1536), dtype=np.float32)
    for i, h in enumerate(range(3, 6)):
        for kk in range(2):
            blk = 1024.0 * (LOG2E * rel[:, kk * 128:(kk + 1) * 128, h].T
                            + 15.0 - C_SHIFT)
            bprime[:, i * 512 + kk * 256: i * 512 + kk * 256 + 256] = blk
    ident = np.eye(128, dtype=np.float16)

    nc = _get_nc()
    in_maps = []
    for c in range(N_CORES):
        b = c // 4
        row0 = (c % 4) * L_PER_CORE
        qT = _pack_qkT(qkv[0, b, row0:row0 + L_PER_CORE], A_PRE)
        kT = _pack_qkT(qkv[1, b, row0:row0 + L_PER_CORE], 1.0)
        vc = qkv[2, b, row0:row0 + L_PER_CORE].reshape(8, 8, 8, 32, DIM)
        win = vc.transpose(0, 2, 1, 3, 4).reshape(64, 2, 128, 6, 32)
        tmp = np.zeros((64, 2, 128, 6, 34), dtype=np.float32)
        tmp[..., :32] = win
        tmp[..., 32] = 1.0
        vext = np.ascontiguousarray(
            tmp.transpose(0, 2, 1, 3, 4).reshape(64, 128, 408).astype(np.float16))
        in_maps.append({
            "qT_c": qT,
            "kT_c": kT,
            "vext_c": vext,
            "biasT_c": biasT,
            "bprime_c": bprime,
            "ident_c": ident,
        })
    res = run_bass_kernel_spmd(nc, in_maps, list(range(N_CORES)), trace=_trace)

    out = np.empty((B, H, W, DIM), dtype=np.float32)
    for c in range(N_CORES):
        b = c // 4
        h0 = (c % 4) * 64
        raw = res.results[c]["out_c"].astype(np.float32)  # [64, 128, 408]
        raw = raw.reshape(64, 128, 2, 6, 34)              # [win, p, qc, h, d]
        num = raw[..., :32]
        den = raw[..., 32]
        o = num / den[..., None]                          # [win, p, qc, h, 32]
        o = o.transpose(0, 2, 1, 3, 4).reshape(64, 256, DIM)  # [win, t, c]
        o = o.reshape(8, 8, 8, 32, DIM).transpose(0, 2, 1, 3, 4)
        out[b, h0:h0 + 64] = o.reshape(64, W, DIM)
    if _trace:
        return out, res
    return out
